# revision 3
# baseline (speedup 1.0000x reference)
"""YOLO-style NMS detection kernel for 8 Trainium2 NeuronCores.

Strategy (data-parallel over the anchor dim, per the spec hint):
  - Host pads anchors 134400 -> 135168 = 8 * 16896 and hands each core a
    [369, 16896] shard.
  - Device (the memory-bound part, ~24.7MB/core of class scores):
    per-anchor max over the 365 class rows.  Channel-major fold
    365->128 on DVE, PE transpose to anchor-major, batched free-dim
    max-reduce -> scores[128, 132] per core.
  - Host (tiny, ~300 boxes): stable top-300 selection (exact
    jax.lax.top_k tie semantics), class argmax + box decode for the
    selected anchors only, and the 300-box greedy NMS.  All host math
    is bit-exact vs the f32 reference (see notes inline).
"""

import sys

for _p in ("/opt/trn_rl_repo",):
    if _p not in sys.path:
        sys.path.insert(0, _p)

import numpy as np

NUM_CLASSES = 365
NUM_ANCHORS = 134400
CONF_THRES = 0.25
IOU_THRES = 0.5
MAX_DET = 300
N_CORES = 8

PAD_ANCHORS = 135168  # 8 * 16896
A_PER_CORE = PAD_ANCHORS // N_CORES  # 16896
GROUPS = A_PER_CORE // 128  # 132
CHUNK = 512  # anchors per DMA chunk
N_CHUNKS = A_PER_CORE // CHUNK  # 33

_COMPILED = {}


def _build_device_program(repeat: int = 1):
    import concourse.bacc as bacc
    import concourse.tile as tile
    from concourse import mybir
    from concourse.masks import make_identity

    f32 = mybir.dt.float32

    nc = bacc.Bacc("TRN2", target_bir_lowering=False, debug=False, num_devices=N_CORES)
    x = nc.dram_tensor("x", [369, A_PER_CORE], f32, kind="ExternalInput")
    s = nc.dram_tensor("s", [128, GROUPS], f32, kind="ExternalOutput")

    with tile.TileContext(nc) as tc:
        with (
            tc.tile_pool(name="const", bufs=1) as const_pool,
            tc.tile_pool(name="inp", bufs=3) as in_pool,
            tc.tile_pool(name="fold", bufs=3) as m_pool,
            tc.tile_pool(name="psum", bufs=6, space="PSUM") as psum_pool,
            tc.tile_pool(name="score", bufs=1) as score_pool,
        ):
            ident = const_pool.tile([128, 128], f32)
            make_identity(nc, ident[:])
            scores_sb = score_pool.tile([128, GROUPS], f32)

            def emit_pass():
                for i in range(N_CHUNKS):
                    c0 = i * CHUNK
                    sl = slice(c0, c0 + CHUNK)
                    t0 = in_pool.tile([128, CHUNK], f32)
                    nc.sync.dma_start(t0[:], x[4:132, sl])
                    t1 = in_pool.tile([128, CHUNK], f32)
                    nc.sync.dma_start(t1[:], x[132:260, sl])
                    t2 = in_pool.tile([109, CHUNK], f32)
                    nc.sync.dma_start(t2[:], x[260:369, sl])

                    m = m_pool.tile([128, CHUNK], f32)
                    nc.vector.tensor_max(m[:], t0[:], t1[:])
                    nc.vector.tensor_max(m[0:109, :], m[0:109, :], t2[:])

                    ptile = psum_pool.tile([128, CHUNK], f32)
                    for j in range(CHUNK // 128):
                        nc.tensor.transpose(
                            ptile[:, j * 128 : (j + 1) * 128],
                            m[:, j * 128 : (j + 1) * 128],
                            ident[:],
                        )
                    nc.vector.tensor_reduce(
                        scores_sb[:, i * 4 : i * 4 + 4],
                        ptile[:].rearrange("p (g c) -> p g c", c=128),
                        axis=mybir.AxisListType.X,
                        op=mybir.AluOpType.max,
                    )

            if repeat == 1:
                emit_pass()
            else:
                with tc.For_i(0, repeat, 1):
                    emit_pass()

            nc.sync.dma_start(s[:], scores_sb[:])

    nc.compile()
    return nc


def _device_scores(full: np.ndarray) -> np.ndarray:
    """full: [369, 134400] f32 -> scores [134400] f32 (max over class rows)."""
    from concourse.bass_utils import run_bass_kernel_spmd

    if "nc" not in _COMPILED:
        _COMPILED["nc"] = _build_device_program()
    nc = _COMPILED["nc"]

    padded = np.zeros((369, PAD_ANCHORS), dtype=np.float32)
    padded[:, :NUM_ANCHORS] = full
    in_maps = [
        {"x": np.ascontiguousarray(padded[:, c * A_PER_CORE : (c + 1) * A_PER_CORE])}
        for c in range(N_CORES)
    ]
    res = run_bass_kernel_spmd(nc, in_maps, list(range(N_CORES)))
    # s[p, g] = score of local anchor 128*g + p
    parts = [res.results[c]["s"].T.reshape(-1) for c in range(N_CORES)]
    return np.concatenate(parts)[:NUM_ANCHORS]


def _postprocess(output: np.ndarray, scores: np.ndarray):
    """Bit-exact host replica of the reference's selection + NMS.

    All box coordinates are trunc'd integer-valued f32 (|v| <= 2560), so
    every IoU intermediate is an exactly-representable integer; the only
    rounding is the final f32 division, identical in numpy and jax.
    """
    full = output[0]  # [369, N]
    masked = np.where(scores >= np.float32(CONF_THRES), scores, np.float32(-1.0))
    masked = masked.astype(np.float32)
    # jax.lax.top_k: descending, ties -> lower index first == stable argsort
    idx = np.argsort(-masked, kind="stable")[:MAX_DET].astype(np.int32)
    top_scores = masked[idx]

    xywh = full[:4, :][:, idx].astype(np.float32)  # [4, K]
    xf = np.float32(1.0)  # IMG_W / IN_W
    half = np.float32(0.5)
    left = np.trunc((xywh[0] - xywh[2] * half) * xf)
    top = np.trunc((xywh[1] - xywh[3] * half) * xf)
    bw = np.trunc(xywh[2] * xf)
    bh = np.trunc(xywh[3] * xf)
    boxes_k = np.stack([left, top, bw, bh], axis=-1).astype(np.float32)  # [K, 4]

    cls_k = np.argmax(full[4:, :][:, idx], axis=0).astype(np.int32)

    keep = top_scores >= np.float32(CONF_THRES)
    ar = np.arange(MAX_DET)
    bs = boxes_k
    for i in range(MAX_DET):
        if not keep[i]:
            continue
        b = bs[i]
        x1 = np.maximum(b[0], bs[:, 0])
        y1 = np.maximum(b[1], bs[:, 1])
        x2 = np.minimum(b[0] + b[2], bs[:, 0] + bs[:, 2])
        y2 = np.minimum(b[1] + b[3], bs[:, 1] + bs[:, 3])
        inter = np.maximum(x2 - x1, np.float32(0.0)) * np.maximum(
            y2 - y1, np.float32(0.0)
        )
        union = b[2] * b[3] + bs[:, 2] * bs[:, 3] - inter
        iou = inter / np.maximum(union, np.float32(1e-9))
        suppress = (iou > np.float32(IOU_THRES)) & (ar > i)
        keep = keep & ~suppress

    return boxes_k, top_scores, cls_k, keep


def kernel(output: np.ndarray):
    output = np.asarray(output, dtype=np.float32)
    assert output.shape == (1, 4 + NUM_CLASSES, NUM_ANCHORS), output.shape
    scores = _device_scores(output[0])
    return _postprocess(output, scores)


# revision 6
# speedup vs baseline: 5.2652x; 5.2652x over previous
"""YOLO-style NMS detection kernel for 8 Trainium2 NeuronCores.

Strategy (data-parallel over the anchor dim, per the spec hint):
  - Host pads anchors 134400 -> 135168 = 8 * 16896 and hands each core a
    [369, 16896] shard.
  - Device (the memory-bound part, ~24.7MB/core of class scores):
    per-anchor max over the 365 class rows.  Channel-major fold
    365->128 on DVE, PE transpose to anchor-major, batched free-dim
    max-reduce -> scores[128, 132] per core.
  - Host (tiny, ~300 boxes): stable top-300 selection (exact
    jax.lax.top_k tie semantics), class argmax + box decode for the
    selected anchors only, and the 300-box greedy NMS.  All host math
    is bit-exact vs the f32 reference (see notes inline).
"""

import sys

for _p in ("/opt/trn_rl_repo",):
    if _p not in sys.path:
        sys.path.insert(0, _p)

import numpy as np

NUM_CLASSES = 365
NUM_ANCHORS = 134400
CONF_THRES = 0.25
IOU_THRES = 0.5
MAX_DET = 300
N_CORES = 8

PAD_ANCHORS = 135168  # 8 * 16896
A_PER_CORE = PAD_ANCHORS // N_CORES  # 16896
GROUPS = A_PER_CORE // 128  # 132
CHUNK = 1408  # anchors per DMA chunk (= 11 groups of 128)
N_CHUNKS = A_PER_CORE // CHUNK  # 12

_COMPILED = {}


def _build_device_program(repeat: int = 1):
    import concourse.bacc as bacc
    import concourse.tile as tile
    from concourse import mybir
    from concourse.masks import make_identity

    f32 = mybir.dt.float32

    nc = bacc.Bacc("TRN2", target_bir_lowering=False, debug=False, num_devices=N_CORES)
    x = nc.dram_tensor("x", [369, A_PER_CORE], f32, kind="ExternalInput")
    s = nc.dram_tensor("s", [128, GROUPS], f32, kind="ExternalOutput")

    gpc = CHUNK // 128  # groups per chunk

    with tile.TileContext(nc) as tc:
        with (
            tc.tile_pool(name="const", bufs=1) as const_pool,
            tc.tile_pool(name="inp", bufs=4) as in_pool,
            tc.tile_pool(name="fold", bufs=4) as m_pool,
            tc.tile_pool(name="psum", bufs=2, space="PSUM") as psum_pool,
            tc.tile_pool(name="score", bufs=1) as score_pool,
        ):
            ident = const_pool.tile([128, 128], f32)
            make_identity(nc, ident[:])
            scores_sb = score_pool.tile([128, GROUPS], f32)

            def emit_pass():
                for i in range(N_CHUNKS):
                    c0 = i * CHUNK
                    sl = slice(c0, c0 + CHUNK)
                    t0 = in_pool.tile([128, CHUNK], f32)
                    t1 = in_pool.tile([128, CHUNK], f32)
                    t2 = in_pool.tile([109, CHUNK], f32)
                    # Spread the load over the three DMA issue paths: the
                    # SWDGE (gpsimd) path sustains far more bandwidth here
                    # than either HWDGE ring, so it carries the bulk (70%).
                    nc.gpsimd.dma_start(t0[:], x[4:132, sl])
                    nc.gpsimd.dma_start(t1[:], x[132:260, sl])
                    nc.sync.dma_start(t2[0:54, :], x[260:314, sl])
                    nc.scalar.dma_start(t2[54:109, :], x[314:369, sl])

                    m = m_pool.tile([128, CHUNK], f32)
                    nc.vector.tensor_max(m[:], t0[:], t1[:])
                    nc.vector.tensor_max(m[0:109, :], m[0:109, :], t2[:])

                    ptile = psum_pool.tile([128, CHUNK], f32)
                    for j in range(gpc):
                        nc.tensor.transpose(
                            ptile[:, j * 128 : (j + 1) * 128],
                            m[:, j * 128 : (j + 1) * 128],
                            ident[:],
                        )
                    nc.vector.tensor_reduce(
                        scores_sb[:, i * gpc : (i + 1) * gpc],
                        ptile[:].rearrange("p (g c) -> p g c", c=128),
                        axis=mybir.AxisListType.X,
                        op=mybir.AluOpType.max,
                    )

            if repeat == 1:
                emit_pass()
            else:
                with tc.For_i(0, repeat, 1):
                    emit_pass()

            nc.sync.dma_start(s[:], scores_sb[:])

    nc.compile()
    return nc


def _device_scores(full: np.ndarray) -> np.ndarray:
    """full: [369, 134400] f32 -> scores [134400] f32 (max over class rows)."""
    from concourse.bass_utils import run_bass_kernel_spmd

    if "nc" not in _COMPILED:
        _COMPILED["nc"] = _build_device_program()
    nc = _COMPILED["nc"]

    padded = np.zeros((369, PAD_ANCHORS), dtype=np.float32)
    padded[:, :NUM_ANCHORS] = full
    in_maps = [
        {"x": np.ascontiguousarray(padded[:, c * A_PER_CORE : (c + 1) * A_PER_CORE])}
        for c in range(N_CORES)
    ]
    res = run_bass_kernel_spmd(nc, in_maps, list(range(N_CORES)))
    # s[p, g] = score of local anchor 128*g + p
    parts = [res.results[c]["s"].T.reshape(-1) for c in range(N_CORES)]
    return np.concatenate(parts)[:NUM_ANCHORS]


def _postprocess(output: np.ndarray, scores: np.ndarray):
    """Bit-exact host replica of the reference's selection + NMS.

    All box coordinates are trunc'd integer-valued f32 (|v| <= 2560), so
    every IoU intermediate is an exactly-representable integer; the only
    rounding is the final f32 division, identical in numpy and jax.
    """
    full = output[0]  # [369, N]
    masked = np.where(scores >= np.float32(CONF_THRES), scores, np.float32(-1.0))
    masked = masked.astype(np.float32)
    # jax.lax.top_k: descending, ties -> lower index first == stable argsort
    idx = np.argsort(-masked, kind="stable")[:MAX_DET].astype(np.int32)
    top_scores = masked[idx]

    xywh = full[:4, :][:, idx].astype(np.float32)  # [4, K]
    xf = np.float32(1.0)  # IMG_W / IN_W
    half = np.float32(0.5)
    left = np.trunc((xywh[0] - xywh[2] * half) * xf)
    top = np.trunc((xywh[1] - xywh[3] * half) * xf)
    bw = np.trunc(xywh[2] * xf)
    bh = np.trunc(xywh[3] * xf)
    boxes_k = np.stack([left, top, bw, bh], axis=-1).astype(np.float32)  # [K, 4]

    cls_k = np.argmax(full[4:, :][:, idx], axis=0).astype(np.int32)

    keep = top_scores >= np.float32(CONF_THRES)
    ar = np.arange(MAX_DET)
    bs = boxes_k
    for i in range(MAX_DET):
        if not keep[i]:
            continue
        b = bs[i]
        x1 = np.maximum(b[0], bs[:, 0])
        y1 = np.maximum(b[1], bs[:, 1])
        x2 = np.minimum(b[0] + b[2], bs[:, 0] + bs[:, 2])
        y2 = np.minimum(b[1] + b[3], bs[:, 1] + bs[:, 3])
        inter = np.maximum(x2 - x1, np.float32(0.0)) * np.maximum(
            y2 - y1, np.float32(0.0)
        )
        union = b[2] * b[3] + bs[:, 2] * bs[:, 3] - inter
        iou = inter / np.maximum(union, np.float32(1e-9))
        suppress = (iou > np.float32(IOU_THRES)) & (ar > i)
        keep = keep & ~suppress

    return boxes_k, top_scores, cls_k, keep


def kernel(output: np.ndarray):
    output = np.asarray(output, dtype=np.float32)
    assert output.shape == (1, 4 + NUM_CLASSES, NUM_ANCHORS), output.shape
    scores = _device_scores(output[0])
    return _postprocess(output, scores)
